# revision 1
# baseline (speedup 1.0000x reference)
"""Chamfer-KL loss kernel for Trainium2 (8 NeuronCores, batch-parallel).

Per core: one batch sample.
  M[i,j] = mu_p[i]@mu_g[j] - 0.5||mu_p[i]||^2 - 0.5||mu_g[j]||^2  (= -dist/2)
computed as a single bf16 matmul with the norm terms folded in as two extra
contraction rows (K=34). Two passes ([i,j] and [j,i] layouts) so both argmax
directions are free-axis scans (DVE max8 + max_index). Indices drive an
indirect-DMA gather of the full fp32 (mu, logvar) rows, and the KL is
computed exactly in fp32 on-chip.
"""

import numpy as np

BS, N, D = 8, 4096, 32
NT = N // 128  # 32 partition tiles
KAUG = D + 2   # 32 features + norm row + ones row

_NC_CACHE = {}


def _build():
    from contextlib import ExitStack

    import concourse.mybir as mybir
    from concourse import bacc
    from concourse.bass import IndirectOffsetOnAxis
    from concourse.tile import TileContext

    f32 = mybir.dt.float32
    bf16 = mybir.dt.bfloat16
    fp16 = mybir.dt.float16
    u32 = mybir.dt.uint32
    AF = mybir.ActivationFunctionType

    nc = bacc.Bacc(None, target_bir_lowering=False)
    xT = nc.dram_tensor("xT", [KAUG, N], bf16, kind="ExternalInput")
    yT = nc.dram_tensor("yT", [KAUG, N], bf16, kind="ExternalInput")
    cat_p = nc.dram_tensor("cat_p", [N, 2 * D], f32, kind="ExternalInput")
    cat_g = nc.dram_tensor("cat_g", [N, 2 * D], f32, kind="ExternalInput")
    loss = nc.dram_tensor("loss", [1, 1], f32, kind="ExternalOutput")

    with TileContext(nc) as tc:
        with ExitStack() as ctx:
            const = ctx.enter_context(tc.tile_pool(name="const", bufs=1))
            stage_pool = ctx.enter_context(tc.tile_pool(name="stage", bufs=4))
            psum_pool = ctx.enter_context(
                tc.tile_pool(name="psum", bufs=2, space="PSUM")
            )
            small = ctx.enter_context(tc.tile_pool(name="small", bufs=4))

            xT_sb = const.tile([KAUG, N], bf16, tag="xT_sb")
            yT_sb = const.tile([KAUG, N], bf16, tag="yT_sb")
            nat_p = const.tile([128, NT, 2 * D], f32, tag="nat_p")
            nat_g = const.tile([128, NT, 2 * D], f32, tag="nat_g")
            # max_index writes its full top-8 row directly; column 0 is the argmax
            args_y = const.tile([128, NT, 8], u32, tag="args_y")
            args_x = const.tile([128, NT, 8], u32, tag="args_x")

            nc.sync.dma_start(out=xT_sb[:, :], in_=xT[:, :])
            nc.sync.dma_start(out=yT_sb[:, :], in_=yT[:, :])
            nc.sync.dma_start(
                out=nat_p[:, :, :],
                in_=cat_p.rearrange("(t p) c -> p t c", p=128),
            )
            nc.sync.dma_start(
                out=nat_g[:, :, :],
                in_=cat_g.rearrange("(t p) c -> p t c", p=128),
            )

            # --- main scans: layout A (stationary=x) then layout B ---
            for stat_sb, mov_sb, args in (
                (xT_sb, yT_sb, args_y),
                (yT_sb, xT_sb, args_x),
            ):
                for t in range(NT):
                    # stage in fp16 (10-bit mantissa: fewer argmin flips than bf16;
                    # same 2-byte 2x TT class): value pass = wide TT-max tree
                    stage = stage_pool.tile([128, N], fp16, tag="stage")
                    for h in range(2):
                        ps = psum_pool.tile([128, 2048], f32, tag="ps")
                        for q in range(4):
                            c = h * 4 + q
                            nc.tensor.matmul(
                                ps[:, q * 512 : (q + 1) * 512],
                                lhsT=stat_sb[:, t * 128 : (t + 1) * 128],
                                rhs=mov_sb[:, c * 512 : (c + 1) * 512],
                                start=True,
                                stop=True,
                            )
                        nc.scalar.copy(
                            out=stage[:, h * 2048 : (h + 1) * 2048], in_=ps[:, :]
                        )
                    # wide binary tree: 3 TT-max ops (2x bf16 mode) instead
                    # of 7 — fewer DVE dispatch gaps
                    h1 = small.tile([128, 2048], fp16, tag="h1")
                    nc.vector.tensor_tensor(
                        h1[:, :],
                        stage[:, 0:2048],
                        stage[:, 2048:4096],
                        op=mybir.AluOpType.max,
                    )
                    acc = small.tile([128, 1024], fp16, tag="acc")
                    nc.vector.tensor_tensor(
                        acc[:, :],
                        h1[:, 0:1024],
                        h1[:, 1024:2048],
                        op=mybir.AluOpType.max,
                    )
                    nc.vector.tensor_tensor(
                        acc[:, 0:512],
                        acc[:, 0:512],
                        acc[:, 512:1024],
                        op=mybir.AluOpType.max,
                    )
                    top8 = small.tile([128, 8], fp16, tag="top8")
                    nc.vector.max(out=top8[:, :], in_=acc[:, 0:512])
                    nc.vector.max_index(args[:, t, :], top8[:, :], stage[:, :])

            # --- gathers: rows of (mu|logvar) at the argmin indices ---
            # One indirect DMA per tile column: offsets [128, 1] gather one
            # row per partition (the layout walrus's indirect lowering
            # expects — a flat [128, NT] offset AP is misinterpreted).
            gath_g = const.tile([128, NT, 2 * D], f32, tag="gath_g")
            gath_p = const.tile([128, NT, 2 * D], f32, tag="gath_p")
            for t in range(NT):
                nc.gpsimd.indirect_dma_start(
                    gath_g[:, t, :],
                    None,
                    cat_g[:, :],
                    IndirectOffsetOnAxis(ap=args_y[:, t, 0:1], axis=0),
                )
                nc.gpsimd.indirect_dma_start(
                    gath_p[:, t, :],
                    None,
                    cat_p[:, :],
                    IndirectOffsetOnAxis(ap=args_x[:, t, 0:1], axis=0),
                )

            # --- exact fp32 KL on gathered rows ---
            # per-side scratch so the two sides' chains interleave (no WAR
            # serialization) and DVE isn't stalled on the ACT exps
            klacc = const.tile([128, NT], f32, tag="klacc")

            def kl_side(mu_pv, lv_pv, mu_ov, lv_ov, first, sfx):
                # S = sum_d (t1 - exp(t1) - (mu_p-mu_o)^2 * exp(-lv_o)),
                # with t1 = lv_p - lv_o.  (the "+1" per dim is folded in later)
                sc1 = const.tile([128, NT, D], f32, tag="sc1" + sfx)
                sc2 = const.tile([128, NT, D], f32, tag="sc2" + sfx)
                sc3 = const.tile([128, NT, D], f32, tag="sc3" + sfx)
                nc.vector.tensor_sub(sc1[:, :, :], lv_pv, lv_ov)
                nc.scalar.activation(sc2[:, :, :], sc1[:, :, :], AF.Exp)
                nc.vector.tensor_sub(sc1[:, :, :], sc1[:, :, :], sc2[:, :, :])
                nc.vector.tensor_sub(sc2[:, :, :], mu_pv, mu_ov)
                nc.scalar.activation(sc2[:, :, :], sc2[:, :, :], AF.Square)
                nc.scalar.activation(sc3[:, :, :], lv_ov, AF.Exp, scale=-1.0)
                nc.vector.tensor_mul(sc2[:, :, :], sc2[:, :, :], sc3[:, :, :])
                nc.vector.tensor_sub(sc1[:, :, :], sc1[:, :, :], sc2[:, :, :])
                if first:
                    nc.vector.reduce_sum(
                        klacc[:, :], sc1[:, :, :], axis=mybir.AxisListType.X
                    )
                else:
                    red = small.tile([128, NT], f32, tag="red")
                    nc.vector.reduce_sum(
                        red[:, :], sc1[:, :, :], axis=mybir.AxisListType.X
                    )
                    nc.vector.tensor_add(klacc[:, :], klacc[:, :], red[:, :])

            # loss_2 side: p = natural preds, o = gathered gts
            kl_side(
                nat_p[:, :, 0:D],
                nat_p[:, :, D : 2 * D],
                gath_g[:, :, 0:D],
                gath_g[:, :, D : 2 * D],
                first=True,
                sfx="a",
            )
            # loss_1 side: p = gathered preds, o = natural gts
            kl_side(
                gath_p[:, :, 0:D],
                gath_p[:, :, D : 2 * D],
                nat_g[:, :, 0:D],
                nat_g[:, :, D : 2 * D],
                first=False,
                sfx="b",
            )
            # fold the two "+ sum_d 1 = +D" constants (one per side)
            nc.vector.tensor_scalar_add(klacc[:, :], klacc[:, :], float(2 * D))

            # partition-sum via ones-vector matmul (exact fp32 in PSUM)
            ones_col = const.tile([128, 1], f32, tag="ones_col")
            nc.vector.memset(ones_col[:, :], 1.0)
            ps_fin = psum_pool.tile([128, 2048], f32, tag="ps")
            nc.tensor.matmul(
                ps_fin[0:1, 0:NT],
                lhsT=ones_col[:, :],
                rhs=klacc[:, :],
                start=True,
                stop=True,
            )
            fin = small.tile([1, 1], f32, tag="fin")
            nc.vector.reduce_sum(
                fin[:, :], ps_fin[0:1, 0:NT], axis=mybir.AxisListType.X
            )
            # loss = 0.5*(l1+l2), each l = -0.5*S  ->  -0.25*(S1+S2)
            nc.vector.tensor_scalar_mul(fin[:, :], fin[:, :], -0.25)
            nc.sync.dma_start(out=loss[:, :], in_=fin[:, :])

    nc.finalize()
    return nc


def _get_nc():
    if "nc" not in _NC_CACHE:
        _NC_CACHE["nc"] = _build()
    return _NC_CACHE["nc"]


def _host_prep(mu_p, lv_p, mu_g, lv_g):
    """Per-sample input marshalling: bf16 transposed/augmented matmul
    operands and the fp32 (mu|logvar) gather tables."""
    import ml_dtypes

    bf16 = ml_dtypes.bfloat16
    x = mu_p.astype(bf16)
    y = mu_g.astype(bf16)
    xf = x.astype(np.float32)
    yf = y.astype(np.float32)
    ax = (-0.5 * np.sum(xf * xf, -1)).astype(bf16)
    ay = (-0.5 * np.sum(yf * yf, -1)).astype(bf16)
    ones = np.ones((N,), bf16)
    xT = np.ascontiguousarray(np.concatenate([x.T, ax[None, :], ones[None, :]], 0))
    yT = np.ascontiguousarray(np.concatenate([y.T, ones[None, :], ay[None, :]], 0))
    cat_p = np.ascontiguousarray(
        np.concatenate([mu_p, lv_p], 1).astype(np.float32)
    )
    cat_g = np.ascontiguousarray(
        np.concatenate([mu_g, lv_g], 1).astype(np.float32)
    )
    return {"xT": xT, "yT": yT, "cat_p": cat_p, "cat_g": cat_g}


def make_in_maps(mu_preds, logvar_preds, mu_gts, logvar_gts):
    mu_preds = np.asarray(mu_preds, dtype=np.float32)
    logvar_preds = np.asarray(logvar_preds, dtype=np.float32)
    mu_gts = np.asarray(mu_gts, dtype=np.float32)
    logvar_gts = np.asarray(logvar_gts, dtype=np.float32)
    return [
        _host_prep(mu_preds[b], logvar_preds[b], mu_gts[b], logvar_gts[b])
        for b in range(BS)
    ]


def run(in_maps, trace=False):
    from concourse.bass_utils import run_bass_kernel_spmd

    nc = _get_nc()
    res = run_bass_kernel_spmd(nc, in_maps, list(range(BS)), trace=trace)
    out = np.array(
        [np.asarray(res.results[b]["loss"]).reshape(()) for b in range(BS)],
        dtype=np.float32,
    )
    return out, res


def kernel(mu_preds, logvar_preds, mu_gts, logvar_gts):
    in_maps = make_in_maps(mu_preds, logvar_preds, mu_gts, logvar_gts)
    out, _ = run(in_maps)
    return out

